# revision 8
# baseline (speedup 1.0000x reference)
"""Trainium2 Bass kernel for nn_GroupFeatureBuilder (segment_reduce).

Shard M=4096 groups across 8 cores (512 each).  All gathers become dense
matmuls against a host-built multiplicity matrix C[m, n]:

  X = [attn_rr | h_hi | h_lo]  fp8e4, DoubleRow pair layout
  E = C@X per 128-group chunk (A part: 2 psum banks per 1024-col half,
  H part: 1 bank), one shared stationary ct slice per contraction pair.

  t1[m]  = <E[m], C[m]>          (DVE dot on bf16 SBUF copy of E)
  ao[m]  = <E[m], min(C,1)-1>
  a_in   = (t1 - t2) * inv_in    (t2, inv_in from host-gathered smalls)
  a_out  = ao * neginv_out
  h_g    = hi_g/16 + lo_g/256    (hi = fp8(h), lo = fp8((h-hi)*16))
  h_glob = host-computed row, broadcast via a 1-contraction matmul.

Schedule: 8 warmup matmuls beat the HAM throttle while the first DMAs
land; a j-major window (U00,U01,U10 + H0,H1 joining mid-window = 8 psum
banks) consumes each (ct_j, a_j) pair as it arrives; the tail runs
unit-major from SBUF.  PSUM banks are freed by scalar-engine copies of E
to SBUF bf16, so bank recycling never waits on the cm DMA; the DVE dot
products then run at 2x rate on bf16 operands.  H3 is emitted last so
the end-of-kernel drain chain is short.
"""

import numpy as np
import ml_dtypes

import concourse.bass as bass
import concourse.bacc as bacc
import concourse.tile as tile
import concourse.mybir as mybir
from concourse.bass_utils import run_bass_kernel_spmd

BF16 = ml_dtypes.bfloat16
FP8 = ml_dtypes.float8_e4m3

N = 2048
D = 256
M = 4096
K = 16
NOBS = 64
NCORES = 8
MLOC = M // NCORES
MCH = MLOC // 128
KP = 8                 # DoubleRow k-pairs
XW = N + 2 * D         # 2560 = [A | h_hi | h_lo]
FOUT = 2 * D + 6
SMW = 84

f32 = mybir.dt.float32
bf16 = mybir.dt.bfloat16
fp8 = mybir.dt.float8e4
OP = mybir.AluOpType
AX = mybir.AxisListType
ACT = mybir.ActivationFunctionType
DR = mybir.MatmulPerfMode.DoubleRow

_NC_CACHE = {}


def _build_nc():
    nc = bacc.Bacc("TRN2", target_bir_lowering=False, debug=False,
                   num_devices=NCORES)

    a_d = nc.declare_dram_parameter("a_x", [128, KP * 2 * XW], fp8,
                                    isOutput=False)
    ct_d = nc.declare_dram_parameter("ct_x", [128, KP * 2 * MLOC], fp8,
                                     isOutput=False)
    cm_d = nc.declare_dram_parameter("cm_x", [128, MCH * N], bf16,
                                     isOutput=False)
    sm_d = nc.declare_dram_parameter("sm_x", [128, MCH * SMW], f32,
                                     isOutput=False)
    hg_d = nc.declare_dram_parameter("hg_x", [1, D], bf16, isOutput=False)
    out_d = nc.declare_dram_parameter("out", [MLOC, FOUT], f32, isOutput=True)

    with tile.TileContext(nc) as tc:
        with (
            tc.tile_pool(name="res", bufs=1) as res,
            tc.tile_pool(name="esb", bufs=4) as esbp,
            tc.tile_pool(name="om", bufs=2) as omp,
            tc.tile_pool(name="junk", bufs=2) as junkp,
            tc.tile_pool(name="outp", bufs=4) as outp,
            tc.tile_pool(name="stats", bufs=1) as statp,
            tc.tile_pool(name="hgsb", bufs=2) as hgsp,
            tc.tile_pool(name="psum_a", bufs=3, space="PSUM") as peA_pool,
            tc.tile_pool(name="psum_h", bufs=2, space="PSUM") as peH_pool,
        ):
            # ---- resident tiles; DMA emission order = arrival priority ----
            ct_all = res.tile([128, KP, 2, MLOC], fp8, tag="ct_all")
            a_all = res.tile([128, KP, 2, XW], fp8, tag="a_all")
            cm_all = res.tile([128, MCH, N], bf16, tag="cm_all")
            sm_all = res.tile([128, MCH, SMW], f32, tag="sm_all")
            hgrow = res.tile([1, D], bf16, tag="hgrow")

            nc.sync.dma_start(out=ct_all[:, 0, :, :], in_=ct_d[:, 0:2 * MLOC])
            nc.sync.dma_start(out=hgrow[:], in_=hg_d[:, :])
            nc.sync.dma_start(out=a_all[:, 0, :, :], in_=a_d[:, 0:2 * XW])
            for j in range(1, KP):
                nc.sync.dma_start(out=ct_all[:, j, :, :],
                                  in_=ct_d[:, j * 2 * MLOC:(j + 1) * 2 * MLOC])
                nc.sync.dma_start(out=a_all[:, j, :, :],
                                  in_=a_d[:, j * 2 * XW:(j + 1) * 2 * XW])
            nc.sync.dma_start(out=sm_all[:], in_=sm_d[:, :])
            for m in range(MCH):
                nc.sync.dma_start(out=cm_all[:, m, :],
                                  in_=cm_d[:, m * N:(m + 1) * N])

            ones_row = res.tile([1, 128], bf16, tag="ones_row")
            nc.vector.memset(ones_row[:], 1.0)
            wsta = res.tile([128, 128], bf16, tag="wsta")
            nc.vector.memset(wsta[:], 1.0)
            wmov = res.tile([128, 512], bf16, tag="wmov")
            nc.vector.memset(wmov[:], 1.0)

            peA = {}
            peH = {}
            out_t = {}
            esb = {}
            oms = {}
            # per-m stats tile: 0 t1h0 | 1 t1h1 | 2 aoh0 | 3 aoh1 | 4 t2 |
            #                   7 t1s
            st = {}
            for m in range(MCH):
                out_t[m] = outp.tile([128, FOUT], f32, tag="out",
                                     name=f"out{m}")
                st[m] = statp.tile([128, 8], f32, tag=f"st{m}", name=f"st{m}")

            # ---- PE warmup (HAM un-throttle) + h_glob broadcast ----------
            # full-contraction matmuls: K=1 warmups don't register as
            # PE-busy for the HAM activity window.
            warm = peH_pool.tile([128, 512], f32, tag="peH", name="warm")
            for _ in range(8):
                nc.tensor.matmul(warm[:], wsta[:], wmov[:],
                                 start=True, stop=True)
            pgb = peH_pool.tile([128, 512], f32, tag="peH", name="pgb")
            nc.tensor.matmul(pgb[:, 0:D], ones_row[:1, :], hgrow[:1, :],
                             start=True, stop=True)
            for m in range(MCH):
                nc.scalar.activation(out_t[m][:, D:2 * D], pgb[:, 0:D],
                                     ACT.Copy)

            def alloc_A(m):
                for h in range(2):
                    peA[(m, h)] = peA_pool.tile([128, 1024], f32, tag="peA",
                                                name=f"peA{m}_{h}")

            def alloc_H(m):
                peH[m] = peH_pool.tile([128, 512], f32, tag="peH",
                                       name=f"peH{m}")

            def emit_A(m, h, j, start, stop):
                for s in range(2):
                    c0 = h * 1024 + s * 512
                    nc.tensor.matmul(
                        peA[(m, h)][:, s * 512:(s + 1) * 512],
                        ct_all[:, j, :, m * 128:(m + 1) * 128],
                        a_all[:, j, :, c0:c0 + 512],
                        start=start, stop=stop, perf_mode=DR)

            def emit_H(m, j, start, stop):
                nc.tensor.matmul(peH[m][:],
                                 ct_all[:, j, :, m * 128:(m + 1) * 128],
                                 a_all[:, j, :, N:XW],
                                 start=start, stop=stop, perf_mode=DR)

            def copy_E(m, h):
                esb[(m, h)] = esbp.tile([128, 1024], bf16, tag="esb",
                                        name=f"esb{m}_{h}")
                nc.scalar.activation(esb[(m, h)][:], peA[(m, h)][:], ACT.Copy)

            def make_om(m):
                oms[m] = omp.tile([128, N], bf16, tag="om", name=f"om{m}")
                nc.vector.tensor_scalar(out=oms[m][:], in0=cm_all[:, m, :],
                                        scalar1=1.0, scalar2=-1.0,
                                        op0=OP.min, op1=OP.add)

            def dots(m, h):
                c0 = h * 1024
                jk = junkp.tile([128, 1024], bf16, tag="jk")
                nc.vector.scalar_tensor_tensor(
                    out=jk[:], in0=esb[(m, h)][:], scalar=1.0,
                    in1=cm_all[:, m, c0:c0 + 1024],
                    op0=OP.mult, op1=OP.mult, accum_out=st[m][:, h:h + 1])
                jk2 = junkp.tile([128, 1024], bf16, tag="jk")
                nc.vector.scalar_tensor_tensor(
                    out=jk2[:], in0=esb[(m, h)][:], scalar=1.0,
                    in1=oms[m][:, c0:c0 + 1024],
                    op0=OP.mult, op1=OP.mult, accum_out=st[m][:, 2 + h:3 + h])

            def drain_H(m):
                hgs = hgsp.tile([128, D], f32, tag="hgs", name=f"hgs{m}")
                nc.scalar.activation(hgs[:], peH[m][:, 0:D], ACT.Copy,
                                     scale=1.0 / K)
                nc.vector.scalar_tensor_tensor(
                    out=out_t[m][:, 0:D], in0=peH[m][:, D:2 * D],
                    scalar=1.0 / (K * 16.0), in1=hgs[:],
                    op0=OP.mult, op1=OP.add)

            def smalls(m):
                ot = out_t[m]
                s = st[m]
                jks = statp.tile([128, 16], f32, tag=f"jks{m}",
                                 name=f"jks{m}")
                nc.vector.tensor_scalar(out=jks[:], in0=sm_all[:, m, 32:48],
                                        scalar1=1.0, scalar2=0.0,
                                        op0=OP.mult, op1=OP.add,
                                        accum_out=s[:, 4:5])
                jk2 = statp.tile([128, 16], f32, tag=f"jk2_{m}",
                                 name=f"jk2_{m}")
                nc.vector.tensor_scalar(out=jk2[:], in0=sm_all[:, m, 0:16],
                                        scalar1=1.0 / K, scalar2=0.0,
                                        op0=OP.mult, op1=OP.add,
                                        accum_out=ot[:, 516:517])
                jk3 = statp.tile([128, 16], f32, tag=f"jk3_{m}",
                                 name=f"jk3_{m}")
                nc.vector.tensor_scalar(out=jk3[:], in0=sm_all[:, m, 64:80],
                                        scalar1=1.0 / (K * NOBS),
                                        scalar2=0.0, op0=OP.mult, op1=OP.add,
                                        accum_out=ot[:, 515:516])
                nc.vector.tensor_reduce(ot[:, 517:518], sm_all[:, m, 16:32],
                                        AX.X, OP.min)
                nc.vector.memset(ot[:, 512:513], float(K) / 3.0)

            def finals(m):
                ot = out_t[m]
                s = st[m]
                # a_in = ((t1h0 + t1h1) - t2) * inv_in
                nc.vector.tensor_sub(s[:, 7:8], s[:, 0:1], s[:, 4:5])
                nc.vector.scalar_tensor_tensor(
                    out=ot[:, 513:514], in0=s[:, 7:8], scalar=s[:, 1:2],
                    in1=sm_all[:, m, 81:82], op0=OP.add, op1=OP.mult)
                # a_out = (aoh0 + aoh1) * neginv_out
                nc.vector.scalar_tensor_tensor(
                    out=ot[:, 514:515], in0=s[:, 2:3], scalar=s[:, 3:4],
                    in1=sm_all[:, m, 80:81], op0=OP.add, op1=OP.mult)

            # ================= window: j-major, paced by a_j DMAs =========
            alloc_A(0)          # peA slots 0, 1
            peA[(1, 0)] = peA_pool.tile([128, 1024], f32, tag="peA",
                                        name="peA1_0")   # slot 2
            alloc_H(0)          # H slot 0 (after warm)
            alloc_H(1)          # H slot 1 (after pgb)
            for j in range(KP):
                emit_A(0, 0, j, j == 0, j == KP - 1)
                emit_A(0, 1, j, j == 0, j == KP - 1)
                if j >= 1:
                    emit_H(0, j, j == 1, False)
                emit_A(1, 0, j, j == 0, j == KP - 1)
                if j >= 2:
                    emit_H(1, j, j == 2, False)

            # DVE: sm-dependent smalls run as soon as sm lands
            for m in range(MCH):
                smalls(m)

            # ---- tail: H leftovers + U11 ---------------------------------
            emit_H(0, 0, False, True)
            peA[(1, 1)] = peA_pool.tile([128, 1024], f32, tag="peA",
                                        name="peA1_1")   # slot 0
            for j in range(KP):
                emit_A(1, 1, j, j == 0, j == KP - 1)
                if j < 2:
                    emit_H(1, j, False, j == 1)

            copy_E(0, 0)
            copy_E(0, 1)
            copy_E(1, 0)
            drain_H(0)
            drain_H(1)
            copy_E(1, 1)

            make_om(0)
            dots(0, 0)
            dots(0, 1)
            make_om(1)
            dots(1, 0)
            dots(1, 1)
            finals(0)
            finals(1)
            nc.sync.dma_start(out=out_d[0:128, :], in_=out_t[0][:])
            nc.sync.dma_start(out=out_d[128:256, :], in_=out_t[1][:])

            # ---- m2 block + H2 -------------------------------------------
            alloc_A(2)          # slots 1, 2
            for j in range(KP):
                emit_A(2, 0, j, j == 0, j == KP - 1)
                emit_A(2, 1, j, j == 0, j == KP - 1)
            alloc_H(2)          # H slot 0
            for j in range(KP):
                emit_H(2, j, j == 0, j == KP - 1)

            copy_E(2, 0)
            copy_E(2, 1)
            make_om(2)
            dots(2, 0)
            dots(2, 1)
            finals(2)
            drain_H(2)
            nc.sync.dma_start(out=out_d[256:384, :], in_=out_t[2][:])

            # ---- m3 block + H3 (H3 last: short end-of-kernel chain) ------
            # unit-major so U30 stops ~1.7us before U31 and its drain
            # overlaps U31's matmuls.
            alloc_A(3)          # slots 0, 1
            for j in range(KP):
                emit_A(3, 0, j, j == 0, j == KP - 1)
            copy_E(3, 0)
            for j in range(KP):
                emit_A(3, 1, j, j == 0, j == KP - 1)
            copy_E(3, 1)
            alloc_H(3)          # H slot 1
            for j in range(KP):
                emit_H(3, j, j == 0, j == KP - 1)

            make_om(3)
            dots(3, 0)
            dots(3, 1)
            finals(3)
            nc.sync.dma_start(out=out_d[384:512, 256:FOUT],
                              in_=out_t[3][:, 256:FOUT])
            drain_H(3)
            nc.sync.dma_start(out=out_d[384:512, 0:256],
                              in_=out_t[3][:, 0:256])
    nc.compile()
    return nc


def _get_nc():
    if "nc" not in _NC_CACHE:
        _NC_CACHE["nc"] = _build_nc()
    return _NC_CACHE["nc"]


def _pair_layout(x):
    """[2048, w] -> [128, 8*2*w]: row p holds [X[2j*128+p,:] | X[(2j+1)*128+p,:]]."""
    w = x.shape[1]
    return np.ascontiguousarray(
        x.reshape(KP, 2, 128, w).transpose(2, 0, 1, 3).reshape(128,
                                                               KP * 2 * w))


def kernel(h, attn_rr, attn_ro, dist_to_goal, clearance, groups):
    h = np.asarray(h, dtype=np.float32)
    attn_rr = np.asarray(attn_rr, dtype=np.float32)
    attn_ro = np.asarray(attn_ro, dtype=np.float32)
    dist_to_goal = np.asarray(dist_to_goal, dtype=np.float32)
    clearance = np.asarray(clearance, dtype=np.float32)
    groups = np.asarray(groups)

    h_hi = h.astype(FP8)
    h_lo = ((h - h_hi.astype(np.float32)) * 16.0).astype(FP8)
    x_full = np.concatenate([attn_rr.astype(FP8), h_hi, h_lo], axis=1)
    a_x = _pair_layout(x_full)
    robs = attn_ro.sum(axis=1, dtype=np.float32)
    diag = np.ascontiguousarray(np.diagonal(attn_rr)).astype(np.float32)
    hg_x = np.ascontiguousarray(
        h.mean(axis=0, dtype=np.float32).astype(BF16).reshape(1, D))

    in_maps = []
    for s in range(NCORES):
        gs = groups[s * MLOC:(s + 1) * MLOC]
        C = np.zeros((MLOC, N), dtype=np.float32)
        np.add.at(C, (np.arange(MLOC)[:, None], gs), 1.0)

        sumcc = (C * C).sum(axis=1)
        nuniq = (C > 0).sum(axis=1).astype(np.float32)
        sm = np.zeros((MLOC, SMW), dtype=np.float32)
        sm[:, 0:16] = dist_to_goal[gs]
        sm[:, 16:32] = clearance[gs]
        sm[:, 32:48] = diag[gs] * C[np.arange(MLOC)[:, None], gs]
        sm[:, 64:80] = robs[gs]
        sm[:, 80] = -1.0 / (K * (N - nuniq))
        sm[:, 81] = 1.0 / np.maximum(K * K - sumcc, 1.0)

        in_maps.append({
            "a_x": a_x,
            "ct_x": _pair_layout(C.T.astype(FP8)),
            "cm_x": np.ascontiguousarray(
                C.astype(BF16).reshape(MCH, 128, N).transpose(1, 0, 2)
                .reshape(128, MCH * N)),
            "sm_x": np.ascontiguousarray(
                sm.reshape(MCH, 128, SMW).transpose(1, 0, 2)
                .reshape(128, MCH * SMW)),
            "hg_x": hg_x,
        })

    nc = _get_nc()
    _NC_CACHE["last_in_maps"] = in_maps
    res = run_bass_kernel_spmd(nc, in_maps, list(range(NCORES)))
    return np.concatenate([res.results[s]["out"] for s in range(NCORES)],
                          axis=0)


# revision 9
# speedup vs baseline: 1.1555x; 1.1555x over previous
"""Trainium2 Bass kernel for nn_GroupFeatureBuilder (segment_reduce).

Shard M=4096 groups across 8 cores (512 each).  All gathers become dense
matmuls against a host-built multiplicity matrix C[m, n]:

  X = [attn_rr | C^T]  fp8e4, DoubleRow pair layout (ct packed as extra
  columns so each pair is one DMA); h = [hi | lo] fp8 in a separate
  param loaded after the window (lo at natural scale so hi and lo
  accumulate into the same PSUM region: h_g needs only one ACT copy).

  E = C@A per 128-group chunk (2 psum banks per 1024-col half)
  t1[m]  = <E[m], C[m]>          (DVE dot on bf16 SBUF copy of E)
  ao[m]  = <E[m], min(C,1)-1>    (om uploaded from host)
  a_in   = (t1 - t2) * inv_in    (t2, inv_in host-gathered per group)
  a_out  = ao * neginv_out
  h_glob = host-computed row, broadcast via a 1-contraction matmul.

Schedule: 8 full-contraction warmup matmuls beat the HAM throttle while
the first DMAs land; a j-major window (U00,U01,U10 in the 1024-wide
psum pool + U11 split across the two 512-wide psum slots = 8 banks)
consumes each a-pair as it arrives, covering ALL of m0/m1's A work.
The tail runs m2-A then m3-A unit-major (so their E copies and DVE dots
start as early as possible), then H0..H3 last (pure-ACT drains, short
end chain).  PSUM banks are freed by scalar-engine copies of E to SBUF
bf16, decoupling bank recycling from the cm/om DMA.
"""

import numpy as np
import ml_dtypes

import concourse.bass as bass
import concourse.bacc as bacc
import concourse.tile as tile
import concourse.mybir as mybir
from concourse.bass_utils import run_bass_kernel_spmd

BF16 = ml_dtypes.bfloat16
FP8 = ml_dtypes.float8_e4m3

N = 2048
D = 256
M = 4096
K = 16
NOBS = 64
NCORES = 8
MLOC = M // NCORES
MCH = MLOC // 128
KP = 8                 # DoubleRow k-pairs
XW = N + MLOC          # 2560 = [A | C^T]
HW = 2 * D             # 512 = [hi | lo]
FOUT = 2 * D + 6
SMW = 8

f32 = mybir.dt.float32
bf16 = mybir.dt.bfloat16
fp8 = mybir.dt.float8e4
OP = mybir.AluOpType
AX = mybir.AxisListType
ACT = mybir.ActivationFunctionType
DR = mybir.MatmulPerfMode.DoubleRow

_NC_CACHE = {}


def _build_nc():
    nc = bacc.Bacc("TRN2", target_bir_lowering=False, debug=False,
                   num_devices=NCORES)

    a_d = nc.declare_dram_parameter("a_x", [128, KP * 2 * XW], fp8,
                                    isOutput=False)
    h_d = nc.declare_dram_parameter("h_x", [128, KP * 2 * HW], fp8,
                                    isOutput=False)
    cm_d = nc.declare_dram_parameter("cm_x", [128, MCH * N], bf16,
                                     isOutput=False)
    om_d = nc.declare_dram_parameter("om_x", [128, MCH * N], bf16,
                                     isOutput=False)
    sm_d = nc.declare_dram_parameter("sm_x", [128, MCH * SMW], f32,
                                     isOutput=False)
    hg_d = nc.declare_dram_parameter("hg_x", [1, D], bf16, isOutput=False)
    out_d = nc.declare_dram_parameter("out", [MLOC, FOUT], f32, isOutput=True)

    with tile.TileContext(nc) as tc:
        with (
            tc.tile_pool(name="res", bufs=1) as res,
            tc.tile_pool(name="esb", bufs=6) as esbp,
            tc.tile_pool(name="junk", bufs=2) as junkp,
            tc.tile_pool(name="outp", bufs=4) as outp,
            tc.tile_pool(name="stats", bufs=1) as statp,
            tc.tile_pool(name="psum_a", bufs=3, space="PSUM") as peA_pool,
            tc.tile_pool(name="psum_h", bufs=2, space="PSUM") as peH_pool,
        ):
            # ---- resident tiles; DMA emission order = arrival priority ----
            a_all = res.tile([128, KP, 2, XW], fp8, tag="a_all")
            h_all = res.tile([128, KP, 2, HW], fp8, tag="h_all")
            cm_all = res.tile([128, MCH, N], bf16, tag="cm_all")
            om_all = res.tile([128, MCH, N], bf16, tag="om_all")
            sm_all = res.tile([128, MCH, SMW], f32, tag="sm_all")
            hgrow = res.tile([1, D], bf16, tag="hgrow")

            nc.sync.dma_start(out=hgrow[:], in_=hg_d[:, :])
            for j in range(KP):
                nc.sync.dma_start(out=a_all[:, j, :, :],
                                  in_=a_d[:, j * 2 * XW:(j + 1) * 2 * XW])
            for m in range(2):
                nc.sync.dma_start(out=cm_all[:, m, :],
                                  in_=cm_d[:, m * N:(m + 1) * N])
                nc.sync.dma_start(out=om_all[:, m, :],
                                  in_=om_d[:, m * N:(m + 1) * N])
            nc.sync.dma_start(out=h_all[:, 0:4, :, :],
                              in_=h_d[:, 0:4 * 2 * HW])
            nc.sync.dma_start(out=h_all[:, 4:8, :, :],
                              in_=h_d[:, 4 * 2 * HW:])
            nc.sync.dma_start(out=sm_all[:], in_=sm_d[:, :])
            for m in range(2, MCH):
                nc.sync.dma_start(out=cm_all[:, m, :],
                                  in_=cm_d[:, m * N:(m + 1) * N])
                nc.sync.dma_start(out=om_all[:, m, :],
                                  in_=om_d[:, m * N:(m + 1) * N])

            ones_row = res.tile([1, 128], bf16, tag="ones_row")
            nc.vector.memset(ones_row[:], 1.0)
            wsta = res.tile([128, 128], bf16, tag="wsta")
            nc.vector.memset(wsta[:], 1.0)
            wmov = res.tile([128, 512], bf16, tag="wmov")
            nc.vector.memset(wmov[:], 1.0)

            peA = {}
            peH = {}
            out_t = {}
            esb = {}
            # per-m stats tile: 0 t1h0 | 1 t1h1 | 2 aoh0 | 3 aoh1 | 7 t1s
            st = {}
            for m in range(MCH):
                out_t[m] = outp.tile([128, FOUT], f32, tag="out",
                                     name=f"out{m}")
                st[m] = statp.tile([128, 8], f32, tag=f"st{m}", name=f"st{m}")
                nc.vector.memset(out_t[m][:, 512:513], float(K) / 3.0)

            def ctsl(m, j):
                c0 = N + m * 128
                return a_all[:, j, :, c0:c0 + 128]

            # ---- PE warmup (HAM un-throttle; K=1 matmuls don't count) ----
            warm = peH_pool.tile([128, 512], f32, tag="peH", name="warm")
            for _ in range(8):
                nc.tensor.matmul(warm[:], wsta[:], wmov[:],
                                 start=True, stop=True)
            # h_glob broadcast
            pgb = peH_pool.tile([128, 512], f32, tag="peH", name="pgb")
            nc.tensor.matmul(pgb[:, 0:D], ones_row[:1, :], hgrow[:1, :],
                             start=True, stop=True)
            for m in range(MCH):
                nc.scalar.activation(out_t[m][:, D:2 * D], pgb[:, 0:D],
                                     ACT.Copy)

            def emit_A(m, h, j, start, stop):
                for s in range(2):
                    c0 = h * 1024 + s * 512
                    nc.tensor.matmul(
                        peA[(m, h)][:, s * 512:(s + 1) * 512],
                        ctsl(m, j), a_all[:, j, :, c0:c0 + 512],
                        start=start, stop=stop, perf_mode=DR)

            def emit_H(m, j):
                # hi and lo accumulate into the same 256-col psum region
                nc.tensor.matmul(peH[m][:, 0:D], ctsl(m, j),
                                 h_all[:, j, :, 0:D],
                                 start=(j == 0), stop=False, perf_mode=DR)
                nc.tensor.matmul(peH[m][:, 0:D], ctsl(m, j),
                                 h_all[:, j, :, D:2 * D],
                                 start=False, stop=(j == KP - 1),
                                 perf_mode=DR)

            def copy_E(m, h):
                esb[(m, h)] = esbp.tile([128, 1024], bf16, tag="esb",
                                        name=f"esb{m}_{h}")
                nc.scalar.activation(esb[(m, h)][:], peA[(m, h)][:], ACT.Copy)

            def dots(m, h):
                c0 = h * 1024
                jk = junkp.tile([128, 1024], bf16, tag="jk")
                nc.vector.scalar_tensor_tensor(
                    out=jk[:], in0=esb[(m, h)][:], scalar=1.0,
                    in1=cm_all[:, m, c0:c0 + 1024],
                    op0=OP.mult, op1=OP.mult, accum_out=st[m][:, h:h + 1])
                jk2 = junkp.tile([128, 1024], bf16, tag="jk")
                nc.vector.scalar_tensor_tensor(
                    out=jk2[:], in0=esb[(m, h)][:], scalar=1.0,
                    in1=om_all[:, m, c0:c0 + 1024],
                    op0=OP.mult, op1=OP.mult, accum_out=st[m][:, 2 + h:3 + h])

            def drain_H(m):
                nc.scalar.activation(out_t[m][:, 0:D], peH[m][:, 0:D],
                                     ACT.Copy, scale=1.0 / K)

            def smcopy(m):
                # sm cols 1..3 = pre-reduced a_obs, ex_dist, ex_clr
                nc.scalar.activation(out_t[m][:, 515:518],
                                     sm_all[:, m, 1:4], ACT.Copy)

            def finals(m):
                ot = out_t[m]
                s = st[m]
                # a_in = ((t1h0 + t1h1) - t2) * inv_in
                nc.vector.tensor_sub(s[:, 7:8], s[:, 0:1],
                                     sm_all[:, m, 0:1])
                nc.vector.scalar_tensor_tensor(
                    out=ot[:, 513:514], in0=s[:, 7:8], scalar=s[:, 1:2],
                    in1=sm_all[:, m, 5:6], op0=OP.add, op1=OP.mult)
                # a_out = (aoh0 + aoh1) * neginv_out
                nc.vector.scalar_tensor_tensor(
                    out=ot[:, 514:515], in0=s[:, 2:3], scalar=s[:, 3:4],
                    in1=sm_all[:, m, 4:5], op0=OP.add, op1=OP.mult)

            # ================= window: j-major, paced by a_j DMAs =========
            # U00, U01, U10 in the 1024-wide pool; U11 split across the two
            # 512-wide psum_h slots => all of m0/m1's A work, 8 banks.
            peA[(0, 0)] = peA_pool.tile([128, 1024], f32, tag="peA",
                                        name="peA0_0")   # slot 0
            peA[(0, 1)] = peA_pool.tile([128, 1024], f32, tag="peA",
                                        name="peA0_1")   # slot 1
            peA[(1, 0)] = peA_pool.tile([128, 1024], f32, tag="peA",
                                        name="peA1_0")   # slot 2
            u11a = peH_pool.tile([128, 512], f32, tag="peH", name="u11a")
            u11b = peH_pool.tile([128, 512], f32, tag="peH", name="u11b")
            for j in range(KP):
                st_j, sp_j = j == 0, j == KP - 1
                emit_A(0, 0, j, st_j, sp_j)
                emit_A(0, 1, j, st_j, sp_j)
                emit_A(1, 0, j, st_j, sp_j)
                nc.tensor.matmul(u11a[:], ctsl(1, j),
                                 a_all[:, j, :, 1024:1536],
                                 start=st_j, stop=sp_j, perf_mode=DR)
                nc.tensor.matmul(u11b[:], ctsl(1, j),
                                 a_all[:, j, :, 1536:2048],
                                 start=st_j, stop=sp_j, perf_mode=DR)

            # E copies free psum slots for the tail blocks
            copy_E(0, 0)
            copy_E(0, 1)
            copy_E(1, 0)
            esb[(1, 1)] = esbp.tile([128, 1024], bf16, tag="esb",
                                    name="esb1_1")
            nc.scalar.activation(esb[(1, 1)][:, 0:512], u11a[:], ACT.Copy)
            nc.scalar.activation(esb[(1, 1)][:, 512:1024], u11b[:], ACT.Copy)

            # ---- m2 block, unit-major ------------------------------------
            peA[(2, 0)] = peA_pool.tile([128, 1024], f32, tag="peA",
                                        name="peA2_0")   # slot 0
            for j in range(KP):
                emit_A(2, 0, j, j == 0, j == KP - 1)
            copy_E(2, 0)
            peA[(2, 1)] = peA_pool.tile([128, 1024], f32, tag="peA",
                                        name="peA2_1")   # slot 1
            for j in range(KP):
                emit_A(2, 1, j, j == 0, j == KP - 1)
            copy_E(2, 1)

            # DVE: dots m0, m1 chase the cm/om arrivals
            dots(0, 0)
            dots(0, 1)
            finals(0)
            dots(1, 0)
            dots(1, 1)
            finals(1)

            # ---- m3 block, unit-major ------------------------------------
            peA[(3, 0)] = peA_pool.tile([128, 1024], f32, tag="peA",
                                        name="peA3_0")   # slot 2
            for j in range(KP):
                emit_A(3, 0, j, j == 0, j == KP - 1)
            copy_E(3, 0)
            peA[(3, 1)] = peA_pool.tile([128, 1024], f32, tag="peA",
                                        name="peA3_1")   # slot 0
            for j in range(KP):
                emit_A(3, 1, j, j == 0, j == KP - 1)
            copy_E(3, 1)

            dots(2, 0)
            dots(2, 1)
            finals(2)

            # ---- H units last: pure-ACT drains, short end chain ----------
            peH[0] = peH_pool.tile([128, 512], f32, tag="peH", name="peH0")
            for j in range(KP):
                emit_H(0, j)
            drain_H(0)
            smcopy(0)
            peH[1] = peH_pool.tile([128, 512], f32, tag="peH", name="peH1")
            for j in range(KP):
                emit_H(1, j)
            drain_H(1)
            smcopy(1)
            nc.sync.dma_start(out=out_d[0:128, :], in_=out_t[0][:])
            nc.sync.dma_start(out=out_d[128:256, :], in_=out_t[1][:])

            dots(3, 0)
            dots(3, 1)
            finals(3)

            peH[2] = peH_pool.tile([128, 512], f32, tag="peH", name="peH2")
            for j in range(KP):
                emit_H(2, j)
            drain_H(2)
            smcopy(2)
            nc.sync.dma_start(out=out_d[256:384, :], in_=out_t[2][:])

            peH[3] = peH_pool.tile([128, 512], f32, tag="peH", name="peH3")
            for j in range(KP):
                emit_H(3, j)
            drain_H(3)
            smcopy(3)
            nc.sync.dma_start(out=out_d[384:512, :], in_=out_t[3][:])
    nc.compile()
    return nc


def _get_nc():
    if "nc" not in _NC_CACHE:
        _NC_CACHE["nc"] = _build_nc()
    return _NC_CACHE["nc"]


def _pair_layout(x):
    """[2048, w] -> [128, 8*2*w]: row p holds [X[2j*128+p,:] | X[(2j+1)*128+p,:]]."""
    w = x.shape[1]
    return np.ascontiguousarray(
        x.reshape(KP, 2, 128, w).transpose(2, 0, 1, 3).reshape(128,
                                                               KP * 2 * w))


def _mchunk(x, w):
    """[MLOC, w] -> [128, MCH*w] grouped by 128-row chunk."""
    return np.ascontiguousarray(
        x.reshape(MCH, 128, w).transpose(1, 0, 2).reshape(128, MCH * w))


def kernel(h, attn_rr, attn_ro, dist_to_goal, clearance, groups):
    h = np.asarray(h, dtype=np.float32)
    attn_rr = np.asarray(attn_rr, dtype=np.float32)
    attn_ro = np.asarray(attn_ro, dtype=np.float32)
    dist_to_goal = np.asarray(dist_to_goal, dtype=np.float32)
    clearance = np.asarray(clearance, dtype=np.float32)
    groups = np.asarray(groups)

    h_hi = h.astype(FP8)
    h_lo = (h - h_hi.astype(np.float32)).astype(FP8)   # natural scale
    h_x = _pair_layout(np.concatenate([h_hi, h_lo], axis=1))
    a8 = attn_rr.astype(FP8)
    robs = attn_ro.sum(axis=1, dtype=np.float32)
    diag = np.ascontiguousarray(np.diagonal(attn_rr)).astype(np.float32)
    hg_x = np.ascontiguousarray(
        h.mean(axis=0, dtype=np.float32).astype(BF16).reshape(1, D))

    in_maps = []
    for s in range(NCORES):
        gs = groups[s * MLOC:(s + 1) * MLOC]
        C = np.zeros((MLOC, N), dtype=np.float32)
        np.add.at(C, (np.arange(MLOC)[:, None], gs), 1.0)

        sumcc = (C * C).sum(axis=1)
        nuniq = (C > 0).sum(axis=1).astype(np.float32)
        sm = np.zeros((MLOC, SMW), dtype=np.float32)
        sm[:, 0] = (diag[gs] * C[np.arange(MLOC)[:, None], gs]).sum(axis=1)
        sm[:, 1] = robs[gs].sum(axis=1) / (K * NOBS)
        sm[:, 2] = dist_to_goal[gs].mean(axis=1)
        sm[:, 3] = clearance[gs].min(axis=1)
        sm[:, 4] = -1.0 / (K * (N - nuniq))
        sm[:, 5] = 1.0 / np.maximum(K * K - sumcc, 1.0)

        a_x = _pair_layout(
            np.concatenate([a8, C.T.astype(FP8)], axis=1))

        in_maps.append({
            "a_x": a_x,
            "h_x": h_x,
            "cm_x": _mchunk(C.astype(BF16), N),
            "om_x": _mchunk((np.minimum(C, 1.0) - 1.0).astype(BF16), N),
            "sm_x": _mchunk(sm, SMW),
            "hg_x": hg_x,
        })

    nc = _get_nc()
    _NC_CACHE["last_in_maps"] = in_maps
    res = run_bass_kernel_spmd(nc, in_maps, list(range(NCORES)))
    return np.concatenate([res.results[s]["out"] for s in range(NCORES)],
                          axis=0)
